# revision 17
# baseline (speedup 1.0000x reference)
"""DeepSeek-V3 TopK router kernel for Trainium2 (8 NeuronCores, data-parallel).

Routing math per token (256 experts, 8 groups of 32):
  scores = sigmoid(logits)                 [exact fp32: 1/(1+exp(-x))]
  biased = scores + correction_bias
  group_score(g) = top2sum(biased[g])
  keep top-4 groups -> mask others to -1e9
  top-8 experts of masked biased scores -> indices (desc order)
  weights = unbiased scores at those indices, normalized, *2.5

All selection math is bitwise-identical to the jax-on-neuron reference:
sigmoid is computed as ACT Exp + exact +1.0 + DVE exact reciprocal, which
matches jax.nn.sigmoid lowered through neuronx-cc bit-for-bit.
"""

import sys

for p in ("/opt/trn_rl_repo", "/opt/trn_rl_repo/concourse"):
    if p not in sys.path:
        sys.path.insert(0, p)

import numpy as np

N_TOKENS = 131072
N_EXPERTS = 256
N_GROUP = 8
GROUP_SIZE = 32
TOP_K = 8
N_CORES = 8
TOK_PER_CORE = N_TOKENS // N_CORES  # 16384
P = 128  # partitions / tokens per tile

_COMPILED = {}


def _build(tokens_per_core: int):
    import concourse.bass as bass  # noqa: F401
    import concourse.mybir as mybir
    import concourse.tile as tile
    from concourse import bacc

    f32 = mybir.dt.float32
    i32 = mybir.dt.int32
    u32 = mybir.dt.uint32
    Alu = mybir.AluOpType
    Act = mybir.ActivationFunctionType
    X = mybir.AxisListType.X

    n_tiles = tokens_per_core // P
    assert tokens_per_core % P == 0
    B = 4  # tiles per batched tail
    assert n_tiles % B == 0

    nc = bacc.Bacc(None, target_bir_lowering=False)
    x = nc.dram_tensor("x", [tokens_per_core, N_EXPERTS], f32, kind="ExternalInput")
    cb = nc.dram_tensor("cb", [1, N_EXPERTS], f32, kind="ExternalInput")
    oi = nc.dram_tensor("oi", [tokens_per_core, TOP_K], i32, kind="ExternalOutput")
    ow = nc.dram_tensor("ow", [tokens_per_core, TOP_K], f32, kind="ExternalOutput")

    with tile.TileContext(nc) as tc:
        with (
            tc.tile_pool(name="const", bufs=1) as cpool,
            tc.tile_pool(name="big", bufs=10) as pool,
            tc.tile_pool(name="small", bufs=10) as spool,
        ):
            bias_row = cpool.tile([128, N_EXPERTS], f32, tag="bias_row")
            bias_rep = cpool.tile([128, N_EXPERTS], f32, tag="bias_rep")
            nc.sync.dma_start(bias_row[:1, :], cb[:])
            nc.gpsimd.partition_broadcast(bias_rep[:], bias_row[:1, :])

            state = {}

            def front(t):
                """load + exact sigmoid + biased scores (DMA/ACT/DVE/Pool)."""
                tok = slice(t * P, (t + 1) * P)
                xt = pool.tile([P, N_EXPERTS], f32, tag="xt")
                nc.sync.dma_start(xt[:], x[tok, :])
                et = pool.tile([P, N_EXPERTS], f32, tag="et")
                nc.scalar.activation(et[:], xt[:], Act.Exp, scale=-1.0)
                ot = pool.tile([P, N_EXPERTS], f32, tag="ot")
                nc.scalar.activation(ot[:], et[:], Act.Identity, bias=1.0)
                st = pool.tile([P, N_EXPERTS], f32, tag="st")
                nc.vector.reciprocal(st[:], ot[:])
                bt = pool.tile([P, N_EXPERTS], f32, tag="bt")
                nc.gpsimd.tensor_add(bt[:], st[:], bias_rep[:])
                state[("st", t)] = st
                state[("bt", t)] = bt

            def groups(t):
                """per-group top-8 (group-scoped, exact for duplicates)."""
                bt = state[("bt", t)]
                b = t % B
                if b == 0:
                    gtop4 = spool.tile([P, 64 * B], f32, tag="gtop4")
                    state[("gtop4", t // B)] = gtop4
                else:
                    gtop4 = state[("gtop4", t // B)]
                for g in range(N_GROUP):
                    nc.vector.max(
                        gtop4[:, b * 64 + g * 8 : b * 64 + (g + 1) * 8],
                        bt[:, g * GROUP_SIZE : (g + 1) * GROUP_SIZE],
                    )

            def gmask(tb):
                """group scores + top-4 group mask for a batch of B tiles."""
                gtop4 = state[("gtop4", tb)]
                g4 = gtop4[:].rearrange("p (b g r) -> p b g r", b=B, g=N_GROUP)
                gs4 = spool.tile([P, 8 * B], f32, tag="gs4")
                gs4v = gs4[:].rearrange("p (b g) -> p b g", b=B)
                nc.vector.tensor_add(gs4v, g4[:, :, :, 0], g4[:, :, :, 1])
                gsort4 = spool.tile([P, 8 * B], f32, tag="gsort4")
                for b in range(B):
                    nc.vector.max(
                        gsort4[:, b * 8 : (b + 1) * 8], gs4[:, b * 8 : (b + 1) * 8]
                    )
                # negm = (gs < 4th-largest) * -1e9, batched over B tiles
                thr = (
                    gsort4[:]
                    .rearrange("p (b g) -> p b g", b=B)[:, :, 3:4]
                    .to_broadcast([P, B, 8])
                )
                negm4 = spool.tile([P, 8 * B], f32, tag="negm4")
                negm4v = negm4[:].rearrange("p (b g) -> p b g", b=B)
                nc.vector.tensor_tensor(negm4v, gs4v, thr, op=Alu.is_lt)
                nc.vector.tensor_scalar_mul(negm4[:], negm4[:], -1.0e9)
                # masked candidates: union of kept groups' top-8 contains the
                # global top-8, so max8 over 64 == max8 over 256 masked.
                cand4 = spool.tile([P, 64 * B], f32, tag="cand4")
                nc.vector.scalar_tensor_tensor(
                    cand4[:].rearrange("p (b g r) -> p b g r", b=B, g=N_GROUP),
                    g4,
                    0.0,
                    negm4v.unsqueeze(3).to_broadcast([P, B, N_GROUP, 8]),
                    op0=Alu.add,
                    op1=Alu.add,
                )
                state[("negm4", tb)] = negm4
                state[("cand4", tb)] = cand4

            def midsel(t):
                """mask, top-8 select, winner marking."""
                st = state[("st", t)]
                bt = state[("bt", t)]
                btg = bt[:].rearrange("p (g k) -> p g k", g=N_GROUP)
                b = t % B
                negm4 = state[("negm4", t // B)]
                cand4 = state[("cand4", t // B)]

                mskd = pool.tile([P, N_EXPERTS], f32, tag="mskd")
                nc.vector.scalar_tensor_tensor(
                    mskd[:].rearrange("p (g k) -> p g k", g=N_GROUP),
                    btg,
                    0.0,
                    negm4[:, b * 8 : (b + 1) * 8]
                    .unsqueeze(2)
                    .to_broadcast([P, N_GROUP, GROUP_SIZE]),
                    op0=Alu.add,
                    op1=Alu.add,
                )

                v8 = spool.tile([P, 8], f32, tag="v8")
                nc.vector.max(v8[:], cand4[:, b * 64 : (b + 1) * 64])
                if b == 0:
                    idxq4 = spool.tile([P, 16 * B], u32, tag="idxq4")
                    state[("idxq4", t // B)] = idxq4
                else:
                    idxq4 = state[("idxq4", t // B)]
                nc.vector.max_index(idxq4[:, b * 8 : (b + 1) * 8], v8[:], mskd[:])

                # mark winners with 4.0; +unbiased -> winners 4+u in (4,5),
                # everything else < 2.4 (or ~ -1e9). u recovered as
                # (4+u)-4, |err| <= 2.4e-7 (weights only, not selection).
                mk2 = pool.tile([P, N_EXPERTS], f32, tag="mk2")
                nc.vector.match_replace(mk2[:], v8[:], mskd[:], 4.0)
                selu = pool.tile([P, N_EXPERTS], f32, tag="selu")
                nc.gpsimd.tensor_add(selu[:], mk2[:], st[:])
                state[("selu", t)] = selu
                if b == B - 1:
                    state.pop(("gtop4", t // B))
                    state.pop(("negm4", t // B))
                    state.pop(("cand4", t // B))

            def tail_batch(tb):
                """unbiased top-8, slot matching, normalize, store: B tiles."""
                idxq4 = state.pop(("idxq4", tb))
                u8s4 = spool.tile([P, 8 * B], f32, tag="u8s4")
                for b in range(B):
                    t = tb * B + b
                    selu = state.pop(("selu", t))
                    state.pop(("st", t))
                    state.pop(("bt", t))
                    nc.vector.max(u8s4[:, b * 8 : (b + 1) * 8], selu[:])
                    nc.vector.max_index(
                        idxq4[:, 8 * B + b * 8 : 8 * B + (b + 1) * 8],
                        u8s4[:, b * 8 : (b + 1) * 8],
                        selu[:],
                    )

                idxf4 = spool.tile([P, 16 * B], f32, tag="idxf4")
                nc.vector.tensor_copy(idxf4[:], idxq4[:])
                # eq[p,b,j,k] = (idx[b,j] == idxu[b,k])
                eq4 = spool.tile([P, 64 * B], f32, tag="eq4")
                eq4v = eq4[:].rearrange("p (b j k) -> p b j k", b=B, j=8)
                ia = idxf4[:, 0 : 8 * B].rearrange("p (b j) -> p b j", b=B)
                iu = idxf4[:, 8 * B : 16 * B].rearrange("p (b k) -> p b k", b=B)
                nc.vector.tensor_tensor(
                    eq4v,
                    ia.unsqueeze(3).to_broadcast([P, B, 8, 8]),
                    iu.unsqueeze(2).to_broadcast([P, B, 8, 8]),
                    op=Alu.is_equal,
                )
                u8sm4 = spool.tile([P, 8 * B], f32, tag="u8sm4")
                nc.vector.tensor_scalar_add(u8sm4[:], u8s4[:], -4.0)
                pr4 = spool.tile([P, 64 * B], f32, tag="pr4")
                pr4v = pr4[:].rearrange("p (b j k) -> p b j k", b=B, j=8)
                nc.vector.tensor_tensor(
                    pr4v,
                    eq4v,
                    u8sm4[:]
                    .rearrange("p (b k) -> p b k", b=B)
                    .unsqueeze(2)
                    .to_broadcast([P, B, 8, 8]),
                    op=Alu.mult,
                )
                u84 = spool.tile([P, 8 * B], f32, tag="u84")
                nc.vector.reduce_sum(
                    u84[:].rearrange("p (b j) -> p b j", b=B), pr4v, axis=X
                )
                den4 = spool.tile([P, B], f32, tag="den4")
                nc.vector.reduce_sum(
                    den4[:], u8sm4[:].rearrange("p (b k) -> p b k", b=B), axis=X
                )
                rden4 = spool.tile([P, B], f32, tag="rden4")
                nc.vector.reciprocal(rden4[:], den4[:])
                w4 = spool.tile([P, 8 * B], f32, tag="w4")
                nc.vector.scalar_tensor_tensor(
                    w4[:].rearrange("p (b j) -> p b j", b=B),
                    u84[:].rearrange("p (b j) -> p b j", b=B),
                    2.5,
                    rden4[:].unsqueeze(2).to_broadcast([P, B, 8]),
                    op0=Alu.mult,
                    op1=Alu.mult,
                )

                for b in range(B):
                    t = tb * B + b
                    tok = slice(t * P, (t + 1) * P)
                    nc.sync.dma_start(
                        oi[tok, :], idxq4[:, b * 8 : (b + 1) * 8].bitcast(i32)
                    )
                    nc.sync.dma_start(ow[tok, :], w4[:, b * 8 : (b + 1) * 8])

            # software-pipelined emission: cross-engine deps are >=1 tile old
            for i in range(n_tiles + 10):
                if i < n_tiles:
                    front(i)
                if 1 <= i <= n_tiles:
                    groups(i - 1)
                if i >= B and (i - B) % B == 0 and (i - B) // B < n_tiles // B:
                    gmask((i - B) // B)
                if 5 <= i < n_tiles + 5:
                    midsel(i - 5)
                if i >= 9 and (i - 9) % B == 0 and (i - 9) // B < n_tiles // B:
                    tail_batch((i - 9) // B)

    nc.finalize()
    return nc


def get_module(tokens_per_core: int = TOK_PER_CORE):
    if tokens_per_core not in _COMPILED:
        _COMPILED[tokens_per_core] = _build(tokens_per_core)
    return _COMPILED[tokens_per_core]


def run(router_logits: np.ndarray, correction_bias: np.ndarray, trace: bool = False):
    """Shard across 8 cores, run, gather. Returns (idx, w[, perf])."""
    from concourse.bass_utils import run_bass_kernel_spmd

    n = router_logits.shape[0]
    tpc = n // N_CORES
    nc = get_module(tpc)
    cb = np.ascontiguousarray(correction_bias.reshape(1, N_EXPERTS), dtype=np.float32)
    in_maps = [
        {
            "x": np.ascontiguousarray(
                router_logits[i * tpc : (i + 1) * tpc], dtype=np.float32
            ),
            "cb": cb,
        }
        for i in range(N_CORES)
    ]
    res = run_bass_kernel_spmd(
        nc, in_maps, core_ids=list(range(N_CORES)), trace=trace
    )
    idx = np.concatenate([r["oi"] for r in res.results], axis=0)
    w = np.concatenate([r["ow"] for r in res.results], axis=0)
    if trace:
        return idx, w, res
    return idx, w


def kernel(router_logits: np.ndarray, correction_bias: np.ndarray):
    idx, w = run(np.asarray(router_logits), np.asarray(correction_bias))
    return idx.astype(np.int32), w.astype(np.float32)


# revision 23
# speedup vs baseline: 1.0131x; 1.0131x over previous
"""DeepSeek-V3 TopK router kernel for Trainium2 (8 NeuronCores, data-parallel).

Routing math per token (256 experts, 8 groups of 32):
  scores = sigmoid(logits)                 [exact fp32: 1/(1+exp(-x))]
  biased = scores + correction_bias
  group_score(g) = top2sum(biased[g])
  keep top-4 groups -> mask others to -1e9
  top-8 experts of masked biased scores -> indices (desc order)
  weights = unbiased scores at those indices, normalized, *2.5

All selection math is bitwise-identical to the jax-on-neuron reference:
sigmoid is computed as ACT Exp + exact +1.0 + DVE exact reciprocal, which
matches jax.nn.sigmoid lowered through neuronx-cc bit-for-bit.
"""

import sys

for p in ("/opt/trn_rl_repo", "/opt/trn_rl_repo/concourse"):
    if p not in sys.path:
        sys.path.insert(0, p)

import numpy as np

N_TOKENS = 131072
N_EXPERTS = 256
N_GROUP = 8
GROUP_SIZE = 32
TOP_K = 8
N_CORES = 8
TOK_PER_CORE = N_TOKENS // N_CORES  # 16384
P = 128  # partitions / tokens per tile

_COMPILED = {}


def _build(tokens_per_core: int):
    import concourse.bass as bass  # noqa: F401
    import concourse.mybir as mybir
    import concourse.tile as tile
    from concourse import bacc

    f32 = mybir.dt.float32
    i32 = mybir.dt.int32
    u32 = mybir.dt.uint32
    Alu = mybir.AluOpType
    Act = mybir.ActivationFunctionType
    X = mybir.AxisListType.X

    n_tiles = tokens_per_core // P
    assert tokens_per_core % P == 0
    B = 4  # tiles per batched tail
    assert n_tiles % B == 0

    nc = bacc.Bacc(None, target_bir_lowering=False)
    x = nc.dram_tensor("x", [tokens_per_core, N_EXPERTS], f32, kind="ExternalInput")
    cb = nc.dram_tensor("cb", [1, N_EXPERTS], f32, kind="ExternalInput")
    oi = nc.dram_tensor("oi", [tokens_per_core, TOP_K], i32, kind="ExternalOutput")
    ow = nc.dram_tensor("ow", [tokens_per_core, TOP_K], f32, kind="ExternalOutput")

    with tile.TileContext(nc) as tc:
        with (
            tc.tile_pool(name="const", bufs=1) as cpool,
            tc.tile_pool(name="big", bufs=10) as pool,
            tc.tile_pool(name="small", bufs=10) as spool,
            tc.tile_pool(name="batch", bufs=3) as bpool,
        ):
            bias_row = cpool.tile([128, N_EXPERTS], f32, tag="bias_row")
            bias_rep = cpool.tile([128, N_EXPERTS], f32, tag="bias_rep")
            nc.sync.dma_start(bias_row[:1, :], cb[:])
            nc.gpsimd.partition_broadcast(bias_rep[:], bias_row[:1, :])

            state = {}

            def front(t):
                """load + exact sigmoid + biased scores (DMA/ACT/DVE/Pool)."""
                tok = slice(t * P, (t + 1) * P)
                xt = pool.tile([P, N_EXPERTS], f32, tag="xt")
                nc.sync.dma_start(xt[:], x[tok, :])
                et = pool.tile([P, N_EXPERTS], f32, tag="et")
                nc.scalar.activation(et[:], xt[:], Act.Exp, scale=-1.0)
                ot = pool.tile([P, N_EXPERTS], f32, tag="ot")
                nc.scalar.activation(ot[:], et[:], Act.Identity, bias=1.0)
                st = pool.tile([P, N_EXPERTS], f32, tag="st")
                nc.vector.reciprocal(st[:], ot[:])
                b = t % B
                if b == 0:
                    bt4 = bpool.tile([P, N_EXPERTS * B], f32, tag="bt4")
                    state[("bt4", t // B)] = bt4
                else:
                    bt4 = state[("bt4", t // B)]
                nc.gpsimd.tensor_add(
                    bt4[:, b * N_EXPERTS : (b + 1) * N_EXPERTS], st[:], bias_rep[:]
                )
                state[("st", t)] = st

            def groups(t):
                """per-group top-8 (group-scoped, exact for duplicates)."""
                bt4 = state[("bt4", t // B)]
                b = t % B
                if b == 0:
                    gtop4 = spool.tile([P, 64 * B], f32, tag="gtop4")
                    state[("gtop4", t // B)] = gtop4
                else:
                    gtop4 = state[("gtop4", t // B)]
                for g in range(N_GROUP):
                    nc.vector.max(
                        gtop4[:, b * 64 + g * 8 : b * 64 + (g + 1) * 8],
                        bt4[
                            :,
                            b * N_EXPERTS
                            + g * GROUP_SIZE : b * N_EXPERTS
                            + (g + 1) * GROUP_SIZE,
                        ],
                    )

            def gmask(tb):
                """group scores + top-4 group mask for a batch of B tiles."""
                gtop4 = state[("gtop4", tb)]
                g4 = gtop4[:].rearrange("p (b g r) -> p b g r", b=B, g=N_GROUP)
                gs4 = spool.tile([P, 8 * B], f32, tag="gs4")
                gs4v = gs4[:].rearrange("p (b g) -> p b g", b=B)
                nc.vector.tensor_add(gs4v, g4[:, :, :, 0], g4[:, :, :, 1])
                gsort4 = spool.tile([P, 8 * B], f32, tag="gsort4")
                for b in range(B):
                    nc.vector.max(
                        gsort4[:, b * 8 : (b + 1) * 8], gs4[:, b * 8 : (b + 1) * 8]
                    )
                # negm = (gs < 4th-largest) * -1e9, batched over B tiles
                thr = (
                    gsort4[:]
                    .rearrange("p (b g) -> p b g", b=B)[:, :, 3:4]
                    .to_broadcast([P, B, 8])
                )
                negm4 = spool.tile([P, 8 * B], f32, tag="negm4")
                negm4v = negm4[:].rearrange("p (b g) -> p b g", b=B)
                nc.vector.tensor_tensor(negm4v, gs4v, thr, op=Alu.is_lt)
                nc.vector.tensor_scalar_mul(negm4[:], negm4[:], -1.0e9)
                # masked candidates: union of kept groups' top-8 contains the
                # global top-8, so max8 over 64 == max8 over 256 masked.
                cand4 = spool.tile([P, 64 * B], f32, tag="cand4")
                nc.vector.scalar_tensor_tensor(
                    cand4[:].rearrange("p (b g r) -> p b g r", b=B, g=N_GROUP),
                    g4,
                    0.0,
                    negm4v.unsqueeze(3).to_broadcast([P, B, N_GROUP, 8]),
                    op0=Alu.add,
                    op1=Alu.add,
                )
                # batched masked scores for the whole B-tile group
                bt4 = state[("bt4", tb)]
                mskd4 = bpool.tile([P, N_EXPERTS * B], f32, tag="mskd4")
                nc.vector.scalar_tensor_tensor(
                    mskd4[:].rearrange(
                        "p (b g k) -> p b g k", b=B, g=N_GROUP
                    ),
                    bt4[:].rearrange("p (b g k) -> p b g k", b=B, g=N_GROUP),
                    0.0,
                    negm4v.unsqueeze(3).to_broadcast([P, B, N_GROUP, GROUP_SIZE]),
                    op0=Alu.add,
                    op1=Alu.add,
                )
                state[("mskd4", tb)] = mskd4
                state[("cand4", tb)] = cand4

            def midsel(t):
                """top-8 select, winner marking."""
                st = state[("st", t)]
                b = t % B
                cand4 = state[("cand4", t // B)]
                mskd = state[("mskd4", t // B)][
                    :, b * N_EXPERTS : (b + 1) * N_EXPERTS
                ]

                v8 = spool.tile([P, 8], f32, tag="v8")
                nc.vector.max(v8[:], cand4[:, b * 64 : (b + 1) * 64])
                if b == 0:
                    idxq4 = spool.tile([P, 16 * B], u32, tag="idxq4")
                    state[("idxq4", t // B)] = idxq4
                else:
                    idxq4 = state[("idxq4", t // B)]
                nc.vector.max_index(idxq4[:, b * 8 : (b + 1) * 8], v8[:], mskd)

                # mark winners with 4.0; +unbiased -> winners 4+u in (4,5),
                # everything else < 2.4 (or ~ -1e9). u recovered as
                # (4+u)-4, |err| <= 2.4e-7 (weights only, not selection).
                mk2 = pool.tile([P, N_EXPERTS], f32, tag="mk2")
                nc.vector.match_replace(mk2[:], v8[:], mskd, 4.0)
                selu = pool.tile([P, N_EXPERTS], f32, tag="selu")
                nc.gpsimd.tensor_add(selu[:], mk2[:], st[:])
                state[("selu", t)] = selu
                if b == B - 1:
                    state.pop(("gtop4", t // B))
                    state.pop(("cand4", t // B))
                    state.pop(("bt4", t // B))
                    state.pop(("mskd4", t // B))

            def tail_batch(tb):
                """unbiased top-8, slot matching, normalize, store: B tiles."""
                idxq4 = state.pop(("idxq4", tb))
                u8s4 = spool.tile([P, 8 * B], f32, tag="u8s4")
                for b in range(B):
                    t = tb * B + b
                    selu = state.pop(("selu", t))
                    state.pop(("st", t))
                    nc.vector.max(u8s4[:, b * 8 : (b + 1) * 8], selu[:])
                    nc.vector.max_index(
                        idxq4[:, 8 * B + b * 8 : 8 * B + (b + 1) * 8],
                        u8s4[:, b * 8 : (b + 1) * 8],
                        selu[:],
                    )

                idxf4 = spool.tile([P, 16 * B], f32, tag="idxf4")
                nc.vector.tensor_copy(idxf4[:], idxq4[:])
                # eq[p,b,j,k] = (idx[b,j] == idxu[b,k])
                eq4 = spool.tile([P, 64 * B], f32, tag="eq4")
                eq4v = eq4[:].rearrange("p (b j k) -> p b j k", b=B, j=8)
                ia = idxf4[:, 0 : 8 * B].rearrange("p (b j) -> p b j", b=B)
                iu = idxf4[:, 8 * B : 16 * B].rearrange("p (b k) -> p b k", b=B)
                nc.vector.tensor_tensor(
                    eq4v,
                    ia.unsqueeze(3).to_broadcast([P, B, 8, 8]),
                    iu.unsqueeze(2).to_broadcast([P, B, 8, 8]),
                    op=Alu.is_equal,
                )
                u8sm4 = spool.tile([P, 8 * B], f32, tag="u8sm4")
                nc.vector.tensor_scalar_add(u8sm4[:], u8s4[:], -4.0)
                pr4 = spool.tile([P, 64 * B], f32, tag="pr4")
                pr4v = pr4[:].rearrange("p (b j k) -> p b j k", b=B, j=8)
                nc.vector.tensor_tensor(
                    pr4v,
                    eq4v,
                    u8sm4[:]
                    .rearrange("p (b k) -> p b k", b=B)
                    .unsqueeze(2)
                    .to_broadcast([P, B, 8, 8]),
                    op=Alu.mult,
                )
                u84 = spool.tile([P, 8 * B], f32, tag="u84")
                nc.vector.reduce_sum(
                    u84[:].rearrange("p (b j) -> p b j", b=B), pr4v, axis=X
                )
                den4 = spool.tile([P, B], f32, tag="den4")
                nc.vector.reduce_sum(
                    den4[:], u8sm4[:].rearrange("p (b k) -> p b k", b=B), axis=X
                )
                rden4 = spool.tile([P, B], f32, tag="rden4")
                nc.vector.reciprocal(rden4[:], den4[:])
                w4 = spool.tile([P, 8 * B], f32, tag="w4")
                nc.vector.scalar_tensor_tensor(
                    w4[:].rearrange("p (b j) -> p b j", b=B),
                    u84[:].rearrange("p (b j) -> p b j", b=B),
                    2.5,
                    rden4[:].unsqueeze(2).to_broadcast([P, B, 8]),
                    op0=Alu.mult,
                    op1=Alu.mult,
                )

                for b in range(B):
                    t = tb * B + b
                    tok = slice(t * P, (t + 1) * P)
                    nc.sync.dma_start(
                        oi[tok, :], idxq4[:, b * 8 : (b + 1) * 8].bitcast(i32)
                    )
                    nc.sync.dma_start(ow[tok, :], w4[:, b * 8 : (b + 1) * 8])

            # software-pipelined emission: cross-engine deps are >=1 tile old
            for i in range(n_tiles + 10):
                if i < n_tiles:
                    front(i)
                if 1 <= i <= n_tiles:
                    groups(i - 1)
                if i >= B and (i - B) % B == 0 and (i - B) // B < n_tiles // B:
                    gmask((i - B) // B)
                if 5 <= i < n_tiles + 5:
                    midsel(i - 5)
                if i >= 9 and (i - 9) % B == 0 and (i - 9) // B < n_tiles // B:
                    tail_batch((i - 9) // B)

    nc.finalize()
    return nc


def get_module(tokens_per_core: int = TOK_PER_CORE):
    if tokens_per_core not in _COMPILED:
        _COMPILED[tokens_per_core] = _build(tokens_per_core)
    return _COMPILED[tokens_per_core]


def run(router_logits: np.ndarray, correction_bias: np.ndarray, trace: bool = False):
    """Shard across 8 cores, run, gather. Returns (idx, w[, perf])."""
    from concourse.bass_utils import run_bass_kernel_spmd

    n = router_logits.shape[0]
    tpc = n // N_CORES
    nc = get_module(tpc)
    cb = np.ascontiguousarray(correction_bias.reshape(1, N_EXPERTS), dtype=np.float32)
    in_maps = [
        {
            "x": np.ascontiguousarray(
                router_logits[i * tpc : (i + 1) * tpc], dtype=np.float32
            ),
            "cb": cb,
        }
        for i in range(N_CORES)
    ]
    res = run_bass_kernel_spmd(
        nc, in_maps, core_ids=list(range(N_CORES)), trace=trace
    )
    idx = np.concatenate([r["oi"] for r in res.results], axis=0)
    w = np.concatenate([r["ow"] for r in res.results], axis=0)
    if trace:
        return idx, w, res
    return idx, w


def kernel(router_logits: np.ndarray, correction_bias: np.ndarray):
    idx, w = run(np.asarray(router_logits), np.asarray(correction_bias))
    return idx.astype(np.int32), w.astype(np.float32)
